# revision 23
# baseline (speedup 1.0000x reference)
"""Trainium2 Bass kernel for nn_FEM_35072702939287 (attention + BN + channel gate).

Math restructuring (validated vs reference):
  A[t,s] = (Wk x + bk)[:,t] . (Wq x + bq)[:,s]
         = [X_aug^T @ H_aug](t,s) + row-const(t) + const
  where X_aug = [X; 1] (65 x TV), H_aug = [G X ; r^T X], G = Wk^T Wq,
  r = Wq^T bk.  Row-constant terms drop under softmax over s.
  We compute A^T tiles [s_block=128, t] so softmax's denominator
  D[t] = sum_s exp(A^T[s,t]) falls out of the PV matmul by augmenting
  the value projection with a ones column.

  The Trans_s conv is folded into V before attention:
  u[s,c] = sum_i v[s,i] Wt[c,i] = X_aug^T @ [WC; bc],  WC = (Wt Wv)^T,
  bc = Wt bv  (the conv bias bt cancels under BN), so the PV
  accumulation directly yields [P2~ ; D] (65 rows x TV) and the old
  post-attention Wt matmuls disappear.  1/D reaches all 64 channel rows
  via a ones-column fp32r replication matmul.  All fp16 matmuls move
  800 columns per instruction (PSUM allows two-bank outputs; staying at
  512 doubles the per-matmul ~219-cycle drain overhead).
  BatchNorm batch stats go through a small AllGather (lower latency
  floor than AllReduce); each core averages the 8 [mean, E[x^2]] pairs.

Sharding: data-parallel over batch N=16 -> 2 batches per core x 8 cores.
"""

import os
import numpy as np

N_CORES = 8
N, C, T, V = 16, 64, 64, 25
TV = T * V            # 1600
IC = 32
NB = N // N_CORES     # batches per core
EPS = 1e-5
NSB = 13              # 12 full 128-row s-blocks + one 64-row tail
SB = [(j * 128, 128) for j in range(12)] + [(1536, 64)]
# phase1 A-psum half-tiles [128, 800] (2 banks); matmul outputs are
# capped at one PSUM bank (512 fp32), so 800-wide tiles split 512+288
HALVES = [(0, 800), (800, 1600)]
CH_H = [(0, 512), (512, 800)]
# phase2: two 800-wide accumulators
CH_P = [(0, 800), (800, 1600)]
# fp32r sub-chunks within an 800-wide psum tile (fp32r moving max is 512)
CH_R = [(0, 512), (512, 800)]
UW = C + 2            # u-projection width: 64 ch + D ones col + pad


def _build(nc, debug=False):
    import concourse.tile as tile
    from concourse import mybir
    from contextlib import ExitStack

    f32 = mybir.dt.float32
    f32r = mybir.dt.float32r
    f16 = mybir.dt.float16
    AF = mybir.ActivationFunctionType
    ALU = mybir.AluOpType
    AX = mybir.AxisListType

    # ---------------- DRAM I/O ----------------
    x_in = nc.dram_tensor("x_in", [NB, C, TV], f32, kind="ExternalInput").ap()
    wq_d = nc.dram_tensor("wq", [IC, C], f32, kind="ExternalInput").ap()
    wk_d = nc.dram_tensor("wk", [IC, C], f32, kind="ExternalInput").ap()
    bk_d = nc.dram_tensor("bk", [IC, 1], f32, kind="ExternalInput").ap()
    wv_d = nc.dram_tensor("wv", [IC, C], f32, kind="ExternalInput").ap()
    bv_d = nc.dram_tensor("bv", [1, IC], f32, kind="ExternalInput").ap()
    wt_d = nc.dram_tensor("wt", [C, IC], f32, kind="ExternalInput").ap()
    gm_d = nc.dram_tensor("gamma", [C, 1], f32, kind="ExternalInput").ap()
    bt2_d = nc.dram_tensor("beta", [C, 1], f32, kind="ExternalInput").ap()
    w1_d = nc.dram_tensor("w1", [C // 16, C], f32, kind="ExternalInput").ap()
    b1_d = nc.dram_tensor("b1", [C // 16, 1], f32, kind="ExternalInput").ap()
    w2_d = nc.dram_tensor("w2", [C, C // 16], f32, kind="ExternalInput").ap()
    b2_d = nc.dram_tensor("b2", [C, 1], f32, kind="ExternalInput").ap()
    out_d = nc.dram_tensor("out", [NB, C, TV], f32, kind="ExternalOutput").ap()
    if debug:
        dbg_ha = nc.dram_tensor("dbg_ha", [C + 1, TV], f32, kind="ExternalOutput").ap()
        dbg_ut = nc.dram_tensor("dbg_ut", [128, NSB, UW], f16, kind="ExternalOutput").ap()
        dbg_eb = nc.dram_tensor("dbg_eb", [128, NSB, TV], f16, kind="ExternalOutput").ap()
        dbg_p2 = nc.dram_tensor("dbg_p2", [NB, C, TV], f32, kind="ExternalOutput").ap()
        dbg_gate = nc.dram_tensor("dbg_gate", [C, NB], f32, kind="ExternalOutput").ap()
        dbg_scsh = nc.dram_tensor("dbg_scsh", [C, 2], f32, kind="ExternalOutput").ap()

    R = C // 16  # 4

    with tile.TileContext(nc) as tc, ExitStack() as ctx:
        consts = ctx.enter_context(tc.tile_pool(name="consts", bufs=1))
        xpool = ctx.enter_context(tc.tile_pool(name="xpool", bufs=2))
        workp = ctx.enter_context(tc.tile_pool(name="workp", bufs=2))
        finp = ctx.enter_context(tc.tile_pool(name="finp", bufs=1))
        statp = ctx.enter_context(tc.tile_pool(name="statp", bufs=1))
        psA = ctx.enter_context(tc.tile_pool(name="psA", bufs=2, space="PSUM"))
        psW = ctx.enter_context(tc.tile_pool(name="psW", bufs=2, space="PSUM"))
        dramp = ctx.enter_context(tc.tile_pool(name="dramp", bufs=1, space="DRAM"))

        # Fire-and-forget collective warmup: initializes the CC channel so
        # the real stats collective later starts with minimal setup cost.
        warm_ci = dramp.tile([1, 2], f32, name="warm_ci")
        warm_co = dramp.tile([N_CORES, 2], f32, name="warm_co", addr_space="Shared")
        nc.gpsimd.collective_compute(
            "AllGather",
            ALU.bypass,
            ins=[warm_ci.opt()],
            outs=[warm_co.opt()],
            replica_groups=[list(range(N_CORES))],
        )

        # ---------------- input DMAs first (sync queue is the x path) ----------
        xa = [None] * NB      # [65, TV] f32 : [X; 1]
        for b in range(NB):
            t = xpool.tile([C + 1, TV], f32, name="xa", tag="xa")
            xa[b] = t
            nc.gpsimd.memset(t[C:C + 1, :], 1.0)
        nc.sync.dma_start(out=xa[0][0:C, :], in_=x_in[0])

        # critical-path weights on the sync queue right after x_in[0]
        wq_sb = consts.tile([IC, C], f32)
        nc.sync.dma_start(out=wq_sb, in_=wq_d)
        wkbk = consts.tile([IC, C + 2], f32)
        nc.vector.memset(wkbk[:, C + 1:C + 2], 0.0)
        nc.sync.dma_start(out=wkbk[:, 0:C], in_=wk_d)
        nc.sync.dma_start(out=wkbk[:, C:C + 1], in_=bk_d)
        wvb = consts.tile([IC, C + 1], f32)        # [Wv | bv]
        nc.sync.dma_start(out=wvb[:, 0:C], in_=wv_d)
        nc.sync.dma_start(out=wvb[:, C:C + 1], in_=bv_d.rearrange("a i -> i a"))
        wt_rep = consts.tile([IC, C], f32)         # Wt^T
        nc.sync.dma_start(out=wt_rep, in_=wt_d.rearrange("c i -> i c"))

        nc.sync.dma_start(out=xa[1][0:C, :], in_=x_in[1])

        # ---------------- constants / weights (gpsimd DMA queue) --------------
        ones1f = consts.tile([C + 1, C], f32)
        nc.vector.memset(ones1f, 1.0)
        ones1 = consts.tile([C + 1, C], f32r)
        nc.vector.tensor_copy(ones1, ones1f)
        # warm up the ACT table (exp set also holds Identity)
        warmz = consts.tile([1, 1], f32)
        nc.vector.memset(warmz, 1.0)
        warmo = consts.tile([1, 1], f32)
        nc.scalar.activation(warmo, warmz, AF.Exp)
        magic = consts.tile([C, 1], mybir.dt.int32)
        nc.vector.memset(magic, 0x5F3759DF)

        w1t = consts.tile([C, R], f32)
        nc.gpsimd.dma_start(out=w1t, in_=w1_d.rearrange("j c -> c j"))
        w2t = consts.tile([R, C], f32)
        nc.gpsimd.dma_start(out=w2t, in_=w2_d.rearrange("c j -> j c"))
        b1_sb = consts.tile([R, 1], f32)
        nc.gpsimd.dma_start(out=b1_sb, in_=b1_d)
        b2_sb = consts.tile([C, 1], f32)
        nc.gpsimd.dma_start(out=b2_sb, in_=b2_d)
        b2n = consts.tile([C, 1], f32)
        nc.vector.tensor_scalar_mul(b2n, b2_sb, -1.0)
        gamma_sb = consts.tile([C, 1], f32)
        nc.gpsimd.dma_start(out=gamma_sb, in_=gm_d)
        beta_sb = consts.tile([C, 1], f32)
        nc.gpsimd.dma_start(out=beta_sb, in_=bt2_d)

        # G^T | r  =  Wq^T @ [Wk | bk]   -> lhsT for the H matmul
        psg = psW.tile([C, C + 2], f32, name="psg", tag="w")
        nc.tensor.matmul(psg, lhsT=wq_sb, rhs=wkbk, start=True, stop=True)
        gr = consts.tile([C, C + 1], f16)
        nc.vector.tensor_copy(gr, psg[:, 0:C + 1])

        # [WC; bc] = [Wv | bv]^T @ Wt^T  -> rhs for the u-projection
        pswc = psW.tile([C + 1, C], f32, name="pswc", tag="w")
        nc.tensor.matmul(pswc, lhsT=wvb, rhs=wt_rep, start=True, stop=True)
        wct_f = consts.tile([C + 1, UW], f32)
        nc.vector.memset(wct_f, 0.0)
        nc.vector.tensor_copy(wct_f[:, 0:C], pswc)
        nc.vector.memset(wct_f[C:C + 1, C:C + 1], 1.0)
        wct = consts.tile([C + 1, UW], f16)
        nc.vector.tensor_copy(wct, wct_f)

        # ---------------- per-batch state ----------------
        xr = [None] * NB      # [65, TV] f16 copy for matmul operands
        ha = [None] * NB      # [65, TV] f16 : [G X; r^T X]
        ut1 = [None] * NB     # [128, 13, 66] f16 : [U | 1 | 0] per s-block
        eb = [None] * NB      # [128, 13, TV] f16 : exp(A^T)
        p2 = [None] * NB      # [64, TV] f32 : p2 (pre-BN, post conv+divide)
        avgs = statp.tile([C, NB], f32)
        stats = statp.tile([C, NB * len(CH_P) * 2, 6], f32)

        def prologue(b):
            t = xa[b]
            tr = xpool.tile([C + 1, TV], f16, name="xr", tag="xr")
            xr[b] = tr
            nc.vector.tensor_copy(tr, t)
            h = xpool.tile([C + 1, TV], f16, name="ha", tag="ha")
            ha[b] = h
            for (t0, t1) in CH_P:
                hps = psW.tile([C + 1, 800], f32, name="hps", tag="w")
                for (c0, c1) in CH_H:
                    nc.tensor.matmul(hps[:, c0:c1], lhsT=gr,
                                     rhs=tr[0:C, t0 + c0:t0 + c1],
                                     start=True, stop=True)
                nc.vector.tensor_copy(h[:, t0:t1], hps)
            nc.vector.reduce_sum(avgs[:, b:b + 1], t[0:C, :], axis=AX.X)
            ut1[b] = xpool.tile([128, NSB, UW], f16, name="ut1", tag="ut1")
            eb[b] = xpool.tile([128, NSB, TV], f16, name="eb", tag="eb")
            p2[b] = xpool.tile([C, TV], f32, name="p2", tag="p2")
            # all u-projection blocks up front (keeps phase1 PE-dense)
            for j, (off, p) in enumerate(SB):
                vps = psW.tile([128, UW], f32, name="vps", tag="w")
                nc.tensor.matmul(vps[0:p, :], lhsT=tr[:, off:off + p],
                                 rhs=wct, start=True, stop=True)
                nc.vector.tensor_copy(ut1[b][0:p, j, :], vps[0:p, :])

        def pv_mm(b, paccs, j, ti):
            off, p = SB[j]
            t0, t1 = CH_P[ti]
            for (c0, c1) in CH_H:
                nc.tensor.matmul(paccs[ti][0:UW, c0:c1],
                                 lhsT=ut1[b][0:p, j, :],
                                 rhs=eb[b][0:p, j, t0 + c0:t0 + c1],
                                 start=(j == 0), stop=(j == NSB - 1))

        def phase1(b, inject=None):
            """A^T block -> exp -> PV accumulation.  The second-half PV of
            block j is emitted after A of block j+1 so its wait on exp(j,h2)
            never blocks the in-order PE queue.  `inject` maps j -> thunk
            emitted at that iteration (fills PE bubbles with foreign work)."""
            paccs = []
            for ti, (t0, t1) in enumerate(CH_P):
                paccs.append(psW.tile([C + 2, 800], f32, name=f"pacc{ti}", tag="w"))
            for j, (off, p) in enumerate(SB):
                for (h0, h1) in HALVES:
                    aps = psA.tile([128, 800], f32, name="aps", tag="aps")
                    for (c0, c1) in CH_H:
                        nc.tensor.matmul(aps[0:p, c0:c1],
                                         lhsT=ha[b][:, off:off + p],
                                         rhs=xr[b][:, h0 + c0:h0 + c1],
                                         start=True, stop=True)
                    nc.scalar.activation(eb[b][0:p, j, h0:h1], aps[0:p, :], AF.Exp)
                if inject and j in inject:
                    inject[j]()
                if j > 0:
                    pv_mm(b, paccs, j - 1, 1)
                pv_mm(b, paccs, j, 0)
            pv_mm(b, paccs, NSB - 1, 1)
            return paccs

        def remainder(b, paccs):
            """[P2~ ; D] -> 1/D replication -> divide -> bn_stats per chunk."""
            pds = []
            for ti, (t0, t1) in enumerate(CH_P):
                pd = workp.tile([C + 1, 800], f32r, name="pd", tag="pd")
                pds.append(pd)
                nc.vector.tensor_copy(pd[0:C + 1, :], paccs[ti][0:C + 1, :])
            for ti, (t0, t1) in enumerate(CH_P):
                w = t1 - t0
                pd = pds[ti]
                dps = psW.tile([C, 800], f32, name="dps", tag="w")
                for (c0, c1) in CH_R:
                    nc.tensor.matmul(dps[:, c0:c1], lhsT=ones1[C:C + 1, :],
                                     rhs=pd[C:C + 1, c0:c1], start=True, stop=True)
                rrep = workp.tile([C, 800], f32, name="rrep", tag="rrep")
                nc.vector.reciprocal_approx_fast(out=rrep[:, 0:w], in_=dps[:, 0:w])
                nc.vector.tensor_mul(p2[b][:, t0:t1], pd[0:C, 0:w].bitcast(f32),
                                     rrep[:, 0:w])
                nc.vector.bn_stats(stats[:, 2 * (b * len(CH_P) + ti), :],
                                   p2[b][:, t0:t0 + 512])
                nc.vector.bn_stats(stats[:, 2 * (b * len(CH_P) + ti) + 1, :],
                                   p2[b][:, t0 + 512:t1])

        prologue(0)
        pa0 = phase1(0, inject={3: lambda: prologue(1)})
        remainder(0, pa0)

        # ---------------- channel gate (hidden under phase1(1)) ----------------
        hps2 = psW.tile([R, NB], f32, name="hps2", tag="w")
        nc.tensor.matmul(hps2, lhsT=w1t, rhs=avgs, start=True, stop=True)
        h_pre = statp.tile([R, NB], f32)
        nc.vector.tensor_scalar(h_pre, hps2, 1.0 / TV, b1_sb,
                                op0=ALU.mult, op1=ALU.add)
        h_sb = statp.tile([R, NB], f32)
        nc.vector.tensor_scalar_max(h_sb, h_pre, 0.0)
        zps = psW.tile([C, NB], f32, name="zps", tag="w")
        nc.tensor.matmul(zps, lhsT=w2t, rhs=h_sb, start=True, stop=True)
        eg = statp.tile([C, NB], f32)
        nc.scalar.activation(eg, zps, AF.Exp, bias=b2n, scale=-1.0)
        gp1 = statp.tile([C, NB], f32)
        nc.vector.tensor_scalar_add(gp1, eg, 1.0)
        gate = statp.tile([C, NB], f32)
        nc.vector.reciprocal(gate, gp1)

        # w_b = gate (.) p2_b can be computed before the stats collective
        wts = [None] * NB

        def w_precompute(b):
            u = workp.tile([C, TV], f32, name="u", tag="u")
            wts[b] = u
            nc.vector.tensor_scalar_mul(u, p2[b], gate[:, b:b + 1])

        w_precompute(0)
        pa1 = phase1(1)
        remainder(1, pa1)
        w_precompute(1)

        # ---------------- BN stats: local -> allgather -> global ----------------
        mv = statp.tile([C, 2], f32)
        nc.vector.bn_aggr(out=mv, in_=stats)
        m2 = statp.tile([C, 1], f32)
        nc.vector.tensor_mul(m2, mv[:, 0:1], mv[:, 0:1])
        sums = statp.tile([C, 2], f32)
        nc.vector.tensor_copy(sums[:, 0:1], mv[:, 0:1])
        nc.vector.tensor_add(sums[:, 1:2], mv[:, 1:2], m2)

        cc_in = dramp.tile([C, 2], f32, name="cc_in")
        cc_out = dramp.tile([N_CORES, C, 2], f32, name="cc_out",
                            addr_space="Shared")
        nc.sync.dma_start(out=cc_in, in_=sums)
        nc.gpsimd.collective_compute(
            "AllGather",
            ALU.bypass,
            ins=[cc_in.opt()],
            outs=[cc_out.opt()],
            replica_groups=[list(range(N_CORES))],
        )
        gs8 = statp.tile([C, 2, N_CORES], f32)
        nc.sync.dma_start(out=gs8, in_=cc_out[:, :, :].rearrange("r c k -> c k r"))

        # Work on 8x-scaled sums to skip the mean/var normalization ops:
        #   v64 = 64*var = (8*gsum1 + 64*eps) - gsum0^2
        #   rstd64 = rsqrt(v64) = rstd/8 ;  sc = (8*gamma)*rstd64
        #   sh = beta - mean*sc = beta - gsum0*(gamma*rstd64)
        gsum = statp.tile([C, 2], f32)
        nc.vector.reduce_sum(gsum, gs8, axis=AX.X)
        mg2 = statp.tile([C, 1], f32)
        nc.vector.tensor_mul(mg2, gsum[:, 0:1], gsum[:, 0:1])
        q8 = statp.tile([C, 1], f32)
        nc.vector.tensor_scalar(q8, gsum[:, 1:2], float(N_CORES),
                                float(N_CORES * N_CORES) * EPS,
                                op0=ALU.mult, op1=ALU.add)
        ve = statp.tile([C, 1], f32)
        nc.vector.tensor_sub(ve, q8, mg2)
        # rstd via fast-inverse-sqrt + 2 Newton steps (no ACT table switch)
        hsh = statp.tile([C, 1], mybir.dt.int32)
        nc.vector.tensor_scalar(hsh, ve.bitcast(mybir.dt.int32), 1, None,
                                op0=ALU.arith_shift_right)
        yi = statp.tile([C, 1], mybir.dt.int32)
        nc.vector.tensor_sub(yi, magic, hsh)
        r1 = statp.tile([C, 1], f32)
        rstd = statp.tile([C, 1], f32)
        t1 = statp.tile([C, 1], f32)
        t3 = statp.tile([C, 1], f32)
        y = yi.bitcast(f32)
        for it, dst in ((0, r1), (1, rstd)):
            nc.vector.tensor_mul(t1, y, y)
            nc.vector.tensor_mul(t1, t1, ve)
            nc.vector.tensor_scalar(t3, t1, -0.5, 1.5, op0=ALU.mult, op1=ALU.add)
            nc.vector.tensor_mul(dst, y, t3)
            y = dst
        scq = statp.tile([C, 1], f32)
        nc.vector.tensor_mul(scq, gamma_sb, rstd)
        sc = statp.tile([C, 1], f32)
        nc.vector.tensor_scalar_mul(sc, scq, float(N_CORES))
        msc = statp.tile([C, 1], f32)
        nc.vector.tensor_mul(msc, gsum[:, 0:1], scq)
        sh = statp.tile([C, 1], f32)
        nc.vector.tensor_sub(sh, beta_sb, msc)

        if debug:
            nc.sync.dma_start(out=dbg_ha, in_=ha[0].bitcast(f32))
            nc.sync.dma_start(out=dbg_ut, in_=ut1[0])
            nc.sync.dma_start(out=dbg_eb, in_=eb[0])
            for _b in range(NB):
                nc.sync.dma_start(out=dbg_p2[_b], in_=p2[_b])
            nc.sync.dma_start(out=dbg_gate, in_=gate)
            nc.sync.dma_start(out=dbg_scsh[:, 0:1], in_=sc)
            nc.sync.dma_start(out=dbg_scsh[:, 1:2], in_=sh)

        # ------------- finalize: out = sc*(gate*p2) + (x + gate*sh) ------------
        # batch 0 on Scalar+Vector, batch 1 on GpSimd, concurrently
        d_0 = statp.tile([C, 1], f32, name="d_0")
        nc.vector.tensor_mul(d_0, gate[:, 0:1], sh)
        d_1 = statp.tile([C, 1], f32, name="d_1")
        nc.vector.tensor_mul(d_1, gate[:, 1:2], sh)
        for b, d_b in ((0, d_0), (1, d_1)):
            x3 = finp.tile([C, TV], f32, name=f"x3_{b}", tag=f"x3_{b}")
            nc.scalar.activation(x3, xa[b][0:C, :], AF.Identity, bias=d_b)
            osb = finp.tile([C, TV], f32, name=f"osb_{b}", tag=f"osb_{b}")
            nc.vector.scalar_tensor_tensor(out=osb, in0=wts[b], scalar=sc,
                                           in1=x3, op0=ALU.mult, op1=ALU.add)
            nc.sync.dma_start(out=out_d[b], in_=osb)


_CACHE = {}


def _get_compiled(debug=False):
    key = ("nc", debug)
    if key in _CACHE:
        return _CACHE[key]
    import concourse.bacc as bacc

    nc = bacc.Bacc("TRN2", target_bir_lowering=False, debug=False,
                   enable_asserts=False, num_devices=N_CORES)
    _build(nc, debug=debug)
    nc.compile()
    _CACHE[key] = nc
    return nc


def _run(inputs, trace=False, debug=False, **kw):
    from concourse import bass_utils

    nc = _get_compiled(debug=debug)
    x = np.ascontiguousarray(np.asarray(inputs["x"], dtype=np.float32))
    x = x.reshape(N, C, TV)
    f = lambda a: np.ascontiguousarray(np.asarray(a, dtype=np.float32))
    common = {
        "wq": f(inputs["Wq"]),
        "wk": f(inputs["Wk"]),
        "bk": f(inputs["bk"]).reshape(IC, 1),
        "wv": f(inputs["Wv"]),
        "bv": f(inputs["bv"]).reshape(1, IC),
        "wt": f(inputs["Wt"]),
        "gamma": f(inputs["gamma"]).reshape(C, 1),
        "beta": f(inputs["beta"]).reshape(C, 1),
        "w1": f(inputs["W1"]),
        "b1": f(inputs["b1"]).reshape(C // 16, 1),
        "w2": f(inputs["W2"]),
        "b2": f(inputs["b2"]).reshape(C, 1),
    }
    in_maps = []
    for c in range(N_CORES):
        m = dict(common)
        m["x_in"] = np.ascontiguousarray(x[c * NB:(c + 1) * NB])
        in_maps.append(m)
    try:
        res = bass_utils.run_bass_kernel_spmd(
            nc, in_maps, core_ids=list(range(N_CORES)), trace=trace, **kw)
    except Exception:
        import time as _time
        _time.sleep(5)
        res = bass_utils.run_bass_kernel_spmd(
            nc, in_maps, core_ids=list(range(N_CORES)), trace=False, **kw)
    out = np.concatenate([res.results[c]["out"] for c in range(N_CORES)], axis=0)
    return out.reshape(N, C, T, V).astype(np.float32), res


def kernel(**inputs):
    return _run(inputs, trace=False)[0]


# revision 28
# speedup vs baseline: 1.0027x; 1.0027x over previous
"""Trainium2 Bass kernel for nn_FEM_35072702939287 (attention + BN + channel gate).

Math restructuring (validated vs reference):
  A[t,s] = (Wk x + bk)[:,t] . (Wq x + bq)[:,s]
         = [X_aug^T @ H_aug](t,s) + row-const(t) + const
  where X_aug = [X; 1] (65 x TV), H_aug = [G X ; r^T X], G = Wk^T Wq,
  r = Wq^T bk.  Row-constant terms drop under softmax over s.
  We compute A^T tiles [s_block=128, t] so softmax's denominator
  D[t] = sum_s exp(A^T[s,t]) falls out of the PV matmul by augmenting
  the value projection with a ones column.

  The Trans_s conv is folded into V before attention:
  u[s,c] = sum_i v[s,i] Wt[c,i] = X_aug^T @ [WC; bc],  WC = (Wt Wv)^T,
  bc = Wt bv  (the conv bias bt cancels under BN), so the PV
  accumulation directly yields [P2~ ; D] (65 rows x TV) and the old
  post-attention Wt matmuls disappear.  1/D reaches all 64 channel rows
  via a ones-column fp32r replication matmul.  All fp16 matmuls move
  800 columns per instruction (PSUM allows two-bank outputs; staying at
  512 doubles the per-matmul ~219-cycle drain overhead).
  BatchNorm batch stats go through a small AllGather (lower latency
  floor than AllReduce); each core averages the 8 [mean, E[x^2]] pairs.

Sharding: data-parallel over batch N=16 -> 2 batches per core x 8 cores.
"""

import os
import numpy as np

N_CORES = 8
N, C, T, V = 16, 64, 64, 25
TV = T * V            # 1600
IC = 32
NB = N // N_CORES     # batches per core
EPS = 1e-5
NSB = 13              # 12 full 128-row s-blocks + one 64-row tail
SB = [(j * 128, 128) for j in range(12)] + [(1536, 64)]
# phase1 A-psum half-tiles [128, 800] (2 banks); matmul outputs are
# capped at one PSUM bank (512 fp32), so 800-wide tiles split 512+288
HALVES = [(0, 800), (800, 1600)]
CH_H = [(0, 512), (512, 800)]
# phase2: two 800-wide accumulators
CH_P = [(0, 800), (800, 1600)]
# fp32r sub-chunks within an 800-wide psum tile (fp32r moving max is 512)
CH_R = [(0, 512), (512, 800)]
UW = C + 2            # u-projection width: 64 ch + D ones col + pad


def _build(nc, debug=False):
    import concourse.tile as tile
    from concourse import mybir
    from contextlib import ExitStack

    f32 = mybir.dt.float32
    f32r = mybir.dt.float32r
    f16 = mybir.dt.float16
    AF = mybir.ActivationFunctionType
    ALU = mybir.AluOpType
    AX = mybir.AxisListType

    # ---------------- DRAM I/O ----------------
    # weights are host-packed into two tensors so startup needs only two
    # small DMAs (each extra DMA costs ~1.3us of issue latency in series)
    x_in = nc.dram_tensor("x_in", [NB, C, TV], f32, kind="ExternalInput").ap()
    wp32_d = nc.dram_tensor("wp32", [IC, 324], f32, kind="ExternalInput").ap()
    wp64_d = nc.dram_tensor("wp64", [C, 7], f32, kind="ExternalInput").ap()
    out_d = nc.dram_tensor("out", [NB, C, TV], f32, kind="ExternalOutput").ap()
    if debug:
        dbg_ha = nc.dram_tensor("dbg_ha", [C + 1, TV], f32, kind="ExternalOutput").ap()
        dbg_ut = nc.dram_tensor("dbg_ut", [128, NSB, UW], f16, kind="ExternalOutput").ap()
        dbg_eb = nc.dram_tensor("dbg_eb", [128, NSB, TV], f16, kind="ExternalOutput").ap()
        dbg_p2 = nc.dram_tensor("dbg_p2", [NB, C, TV], f32, kind="ExternalOutput").ap()
        dbg_gate = nc.dram_tensor("dbg_gate", [C, NB], f32, kind="ExternalOutput").ap()
        dbg_scsh = nc.dram_tensor("dbg_scsh", [C, 2], f32, kind="ExternalOutput").ap()

    R = C // 16  # 4

    with tile.TileContext(nc) as tc, ExitStack() as ctx:
        consts = ctx.enter_context(tc.tile_pool(name="consts", bufs=1))
        xpool = ctx.enter_context(tc.tile_pool(name="xpool", bufs=2))
        workp = ctx.enter_context(tc.tile_pool(name="workp", bufs=2))
        finp = ctx.enter_context(tc.tile_pool(name="finp", bufs=1))
        statp = ctx.enter_context(tc.tile_pool(name="statp", bufs=1))
        psA = ctx.enter_context(tc.tile_pool(name="psA", bufs=2, space="PSUM"))
        psW = ctx.enter_context(tc.tile_pool(name="psW", bufs=2, space="PSUM"))
        dramp = ctx.enter_context(tc.tile_pool(name="dramp", bufs=1, space="DRAM"))

        # Fire-and-forget collective warmup: initializes the CC channel so
        # the real stats collective later starts with minimal setup cost.
        warm_ci = dramp.tile([1, 2], f32, name="warm_ci")
        warm_co = dramp.tile([N_CORES, 2], f32, name="warm_co", addr_space="Shared")
        nc.gpsimd.collective_compute(
            "AllGather",
            ALU.bypass,
            ins=[warm_ci.opt()],
            outs=[warm_co.opt()],
            replica_groups=[list(range(N_CORES))],
        )

        # ---------------- input DMAs first (sync queue is the x path) ----------
        xa = [None] * NB      # [65, TV] f32 : [X; 1]
        for b in range(NB):
            t = xpool.tile([C + 1, TV], f32, name="xa", tag="xa")
            xa[b] = t
            nc.gpsimd.memset(t[C:C + 1, :], 1.0)
        nc.sync.dma_start(out=xa[0][0:C, :], in_=x_in[0])

        # two packed weight DMAs on the sync queue right after x_in[0]
        wpack = consts.tile([IC, 324], f32)
        nc.sync.dma_start(out=wpack, in_=wp32_d)
        wpk2 = consts.tile([C, 7], f32)
        nc.sync.dma_start(out=wpk2, in_=wp64_d)
        nc.sync.dma_start(out=xa[1][0:C, :], in_=x_in[1])

        # views into the packed weights
        wq_sb = wpack[:, 0:C]
        wkbk = wpack[:, C:2 * C + 2]               # [Wk | bk | 0]
        wvb = wpack[:, 2 * C + 2:3 * C + 3]        # [Wv | bv]
        wt_rep = wpack[:, 3 * C + 3:4 * C + 3]     # Wt^T
        w2t = wpack[0:R, 4 * C + 3:5 * C + 3]      # W2^T (rows 0:4)
        b1_sb = wpack[0:R, 5 * C + 3:5 * C + 4]
        w1t = wpk2[:, 0:R]                         # W1^T
        gamma_sb = wpk2[:, R:R + 1]
        beta_sb = wpk2[:, R + 1:R + 2]
        b2_sb = wpk2[:, R + 2:R + 3]

        ones1f = consts.tile([C + 1, C], f32)
        nc.vector.memset(ones1f, 1.0)
        ones1 = consts.tile([C + 1, C], f32r)
        nc.vector.tensor_copy(ones1, ones1f)
        # warm up the ACT table (exp set also holds Identity)
        warmz = consts.tile([1, 1], f32)
        nc.vector.memset(warmz, 1.0)
        warmo = consts.tile([1, 1], f32)
        nc.scalar.activation(warmo, warmz, AF.Exp)
        magic = consts.tile([C, 1], mybir.dt.int32)
        nc.vector.memset(magic, 0x5F3759DF)
        b2n = consts.tile([C, 1], f32)
        nc.vector.tensor_scalar_mul(b2n, b2_sb, -1.0)

        # G^T | r  =  Wq^T @ [Wk | bk]   -> lhsT for the H matmul
        psg = psW.tile([C, C + 2], f32, name="psg", tag="w")
        nc.tensor.matmul(psg, lhsT=wq_sb, rhs=wkbk, start=True, stop=True)
        gr = consts.tile([C, C + 1], f16)
        nc.vector.tensor_copy(gr, psg[:, 0:C + 1])

        # [WC; bc] = [Wv | bv]^T @ Wt^T  -> rhs for the u-projection
        pswc = psW.tile([C + 1, C], f32, name="pswc", tag="w")
        nc.tensor.matmul(pswc, lhsT=wvb, rhs=wt_rep, start=True, stop=True)
        wct_f = consts.tile([C + 1, UW], f32)
        nc.vector.memset(wct_f, 0.0)
        nc.vector.tensor_copy(wct_f[:, 0:C], pswc)
        nc.vector.memset(wct_f[C:C + 1, C:C + 1], 1.0)
        wct = consts.tile([C + 1, UW], f16)
        nc.vector.tensor_copy(wct, wct_f)

        # ---------------- per-batch state ----------------
        xr = [None] * NB      # [65, TV] f16 copy for matmul operands
        ha = [None] * NB      # [65, TV] f16 : [G X; r^T X]
        ut1 = [None] * NB     # [128, 13, 66] f16 : [U | 1 | 0] per s-block
        eb = [None] * NB      # [128, 13, TV] f16 : exp(A^T)
        p2 = [None] * NB      # [64, TV] f32 : p2 (pre-BN, post conv+divide)
        avgs = statp.tile([C, NB], f32)
        stats = statp.tile([C, NB * len(CH_P) * 2, 6], f32)

        def prologue(b, cast_on_scalar=False):
            t = xa[b]
            tr = xpool.tile([C + 1, TV], f16, name="xr", tag="xr")
            xr[b] = tr
            if cast_on_scalar:
                # DVE is the startup bottleneck; ScE is idle before phase1
                nc.scalar.activation(tr, t, AF.Identity)
            else:
                nc.vector.tensor_copy(tr, t)
            h = xpool.tile([C + 1, TV], f16, name="ha", tag="ha")
            ha[b] = h
            for (t0, t1) in CH_P:
                hps = psW.tile([C + 1, 800], f32, name="hps", tag="w")
                for (c0, c1) in CH_H:
                    nc.tensor.matmul(hps[:, c0:c1], lhsT=gr,
                                     rhs=tr[0:C, t0 + c0:t0 + c1],
                                     start=True, stop=True)
                nc.vector.tensor_copy(h[:, t0:t1], hps)
            ut1[b] = xpool.tile([128, NSB, UW], f16, name="ut1", tag="ut1")
            eb[b] = xpool.tile([128, NSB, TV], f16, name="eb", tag="eb")
            p2[b] = xpool.tile([C, TV], f32, name="p2", tag="p2")
            # all u-projection blocks up front (keeps phase1 PE-dense)
            for j, (off, p) in enumerate(SB):
                vps = psW.tile([128, UW], f32, name="vps", tag="w")
                nc.tensor.matmul(vps[0:p, :], lhsT=tr[:, off:off + p],
                                 rhs=wct, start=True, stop=True)
                nc.vector.tensor_copy(ut1[b][0:p, j, :], vps[0:p, :])
            nc.vector.reduce_sum(avgs[:, b:b + 1], t[0:C, :], axis=AX.X)

        def pv_mm(b, paccs, j, ti):
            off, p = SB[j]
            t0, t1 = CH_P[ti]
            for (c0, c1) in CH_H:
                nc.tensor.matmul(paccs[ti][0:UW, c0:c1],
                                 lhsT=ut1[b][0:p, j, :],
                                 rhs=eb[b][0:p, j, t0 + c0:t0 + c1],
                                 start=(j == 0), stop=(j == NSB - 1))

        def phase1(b, inject=None):
            """A^T block -> exp -> PV accumulation.  The second-half PV of
            block j is emitted after A of block j+1 so its wait on exp(j,h2)
            never blocks the in-order PE queue.  `inject` maps j -> thunk
            emitted at that iteration (fills PE bubbles with foreign work)."""
            paccs = []
            for ti, (t0, t1) in enumerate(CH_P):
                paccs.append(psW.tile([C + 2, 800], f32, name=f"pacc{ti}", tag="w"))
            for j, (off, p) in enumerate(SB):
                for (h0, h1) in HALVES:
                    aps = psA.tile([128, 800], f32, name="aps", tag="aps")
                    for (c0, c1) in CH_H:
                        nc.tensor.matmul(aps[0:p, c0:c1],
                                         lhsT=ha[b][:, off:off + p],
                                         rhs=xr[b][:, h0 + c0:h0 + c1],
                                         start=True, stop=True)
                    nc.scalar.activation(eb[b][0:p, j, h0:h1], aps[0:p, :], AF.Exp)
                if inject and j in inject:
                    inject[j]()
                if j > 0:
                    pv_mm(b, paccs, j - 1, 1)
                pv_mm(b, paccs, j, 0)
            pv_mm(b, paccs, NSB - 1, 1)
            return paccs

        def remainder(b, paccs):
            """[P2~ ; D] -> 1/D replication -> divide -> bn_stats per chunk."""
            pds = []
            for ti, (t0, t1) in enumerate(CH_P):
                pd = workp.tile([C + 1, 800], f32r, name="pd", tag="pd")
                pds.append(pd)
                nc.vector.tensor_copy(pd[0:C + 1, :], paccs[ti][0:C + 1, :])
            for ti, (t0, t1) in enumerate(CH_P):
                w = t1 - t0
                pd = pds[ti]
                dps = psW.tile([C, 800], f32, name="dps", tag="w")
                for (c0, c1) in CH_R:
                    nc.tensor.matmul(dps[:, c0:c1], lhsT=ones1[C:C + 1, :],
                                     rhs=pd[C:C + 1, c0:c1], start=True, stop=True)
                rrep = workp.tile([C, 800], f32, name="rrep", tag="rrep")
                nc.vector.reciprocal_approx_fast(out=rrep[:, 0:w], in_=dps[:, 0:w])
                nc.vector.tensor_mul(p2[b][:, t0:t1], pd[0:C, 0:w].bitcast(f32),
                                     rrep[:, 0:w])
                nc.vector.bn_stats(stats[:, 2 * (b * len(CH_P) + ti), :],
                                   p2[b][:, t0:t0 + 512])
                nc.vector.bn_stats(stats[:, 2 * (b * len(CH_P) + ti) + 1, :],
                                   p2[b][:, t0 + 512:t1])

        prologue(0, cast_on_scalar=True)
        pa0 = phase1(0, inject={3: lambda: prologue(1)})
        remainder(0, pa0)

        # ---------------- channel gate (hidden under phase1(1)) ----------------
        hps2 = psW.tile([R, NB], f32, name="hps2", tag="w")
        nc.tensor.matmul(hps2, lhsT=w1t, rhs=avgs, start=True, stop=True)
        h_pre = statp.tile([R, NB], f32)
        nc.vector.tensor_scalar(h_pre, hps2, 1.0 / TV, b1_sb,
                                op0=ALU.mult, op1=ALU.add)
        h_sb = statp.tile([R, NB], f32)
        nc.vector.tensor_scalar_max(h_sb, h_pre, 0.0)
        zps = psW.tile([C, NB], f32, name="zps", tag="w")
        nc.tensor.matmul(zps, lhsT=w2t, rhs=h_sb, start=True, stop=True)
        eg = statp.tile([C, NB], f32)
        nc.scalar.activation(eg, zps, AF.Exp, bias=b2n, scale=-1.0)
        gp1 = statp.tile([C, NB], f32)
        nc.vector.tensor_scalar_add(gp1, eg, 1.0)
        gate = statp.tile([C, NB], f32)
        nc.vector.reciprocal(gate, gp1)

        # w_b = gate (.) p2_b can be computed before the stats collective
        wts = [None] * NB

        def w_precompute(b):
            u = workp.tile([C, TV], f32, name="u", tag="u")
            wts[b] = u
            nc.vector.tensor_scalar_mul(u, p2[b], gate[:, b:b + 1])

        w_precompute(0)
        pa1 = phase1(1)
        remainder(1, pa1)
        w_precompute(1)

        # ---------------- BN stats: local -> allgather -> global ----------------
        mv = statp.tile([C, 2], f32)
        nc.vector.bn_aggr(out=mv, in_=stats)
        m2 = statp.tile([C, 1], f32)
        nc.vector.tensor_mul(m2, mv[:, 0:1], mv[:, 0:1])
        sums = statp.tile([C, 2], f32)
        nc.vector.tensor_copy(sums[:, 0:1], mv[:, 0:1])
        nc.vector.tensor_add(sums[:, 1:2], mv[:, 1:2], m2)

        cc_in = dramp.tile([C, 2], f32, name="cc_in")
        cc_out = dramp.tile([N_CORES, C, 2], f32, name="cc_out",
                            addr_space="Shared")
        nc.sync.dma_start(out=cc_in, in_=sums)
        nc.gpsimd.collective_compute(
            "AllGather",
            ALU.bypass,
            ins=[cc_in.opt()],
            outs=[cc_out.opt()],
            replica_groups=[list(range(N_CORES))],
        )
        gs8 = statp.tile([C, 2, N_CORES], f32)
        nc.sync.dma_start(out=gs8, in_=cc_out[:, :, :].rearrange("r c k -> c k r"))

        # Work on 8x-scaled sums to skip the mean/var normalization ops:
        #   v64 = 64*var = (8*gsum1 + 64*eps) - gsum0^2
        #   rstd64 = rsqrt(v64) = rstd/8 ;  sc = (8*gamma)*rstd64
        #   sh = beta - mean*sc = beta - gsum0*(gamma*rstd64)
        gsum = statp.tile([C, 2], f32)
        nc.vector.reduce_sum(gsum, gs8, axis=AX.X)
        mg2 = statp.tile([C, 1], f32)
        nc.vector.tensor_mul(mg2, gsum[:, 0:1], gsum[:, 0:1])
        q8 = statp.tile([C, 1], f32)
        nc.vector.tensor_scalar(q8, gsum[:, 1:2], float(N_CORES),
                                float(N_CORES * N_CORES) * EPS,
                                op0=ALU.mult, op1=ALU.add)
        ve = statp.tile([C, 1], f32)
        nc.vector.tensor_sub(ve, q8, mg2)
        # rstd via fast-inverse-sqrt + 2 Newton steps (no ACT table switch)
        hsh = statp.tile([C, 1], mybir.dt.int32)
        nc.vector.tensor_scalar(hsh, ve.bitcast(mybir.dt.int32), 1, None,
                                op0=ALU.arith_shift_right)
        yi = statp.tile([C, 1], mybir.dt.int32)
        nc.vector.tensor_sub(yi, magic, hsh)
        r1 = statp.tile([C, 1], f32)
        rstd = statp.tile([C, 1], f32)
        t1 = statp.tile([C, 1], f32)
        t3 = statp.tile([C, 1], f32)
        y = yi.bitcast(f32)
        for it, dst in ((0, r1), (1, rstd)):
            nc.vector.tensor_mul(t1, y, y)
            nc.vector.tensor_mul(t1, t1, ve)
            nc.vector.tensor_scalar(t3, t1, -0.5, 1.5, op0=ALU.mult, op1=ALU.add)
            nc.vector.tensor_mul(dst, y, t3)
            y = dst
        scq = statp.tile([C, 1], f32)
        nc.vector.tensor_mul(scq, gamma_sb, rstd)
        sc = statp.tile([C, 1], f32)
        nc.vector.tensor_scalar_mul(sc, scq, float(N_CORES))
        msc = statp.tile([C, 1], f32)
        nc.vector.tensor_mul(msc, gsum[:, 0:1], scq)
        sh = statp.tile([C, 1], f32)
        nc.vector.tensor_sub(sh, beta_sb, msc)

        if debug:
            nc.sync.dma_start(out=dbg_ha, in_=ha[0].bitcast(f32))
            nc.sync.dma_start(out=dbg_ut, in_=ut1[0])
            nc.sync.dma_start(out=dbg_eb, in_=eb[0])
            for _b in range(NB):
                nc.sync.dma_start(out=dbg_p2[_b], in_=p2[_b])
            nc.sync.dma_start(out=dbg_gate, in_=gate)
            nc.sync.dma_start(out=dbg_scsh[:, 0:1], in_=sc)
            nc.sync.dma_start(out=dbg_scsh[:, 1:2], in_=sh)

        # ------------- finalize: out = sc*(gate*p2) + (x + gate*sh) ------------
        # batch 0 on Scalar+Vector, batch 1 on GpSimd, concurrently
        d_0 = statp.tile([C, 1], f32, name="d_0")
        nc.vector.tensor_mul(d_0, gate[:, 0:1], sh)
        d_1 = statp.tile([C, 1], f32, name="d_1")
        nc.vector.tensor_mul(d_1, gate[:, 1:2], sh)
        for b, d_b in ((0, d_0), (1, d_1)):
            x3 = finp.tile([C, TV], f32, name=f"x3_{b}", tag=f"x3_{b}")
            nc.scalar.activation(x3, xa[b][0:C, :], AF.Identity, bias=d_b)
            osb = finp.tile([C, TV], f32, name=f"osb_{b}", tag=f"osb_{b}")
            nc.vector.scalar_tensor_tensor(out=osb, in0=wts[b], scalar=sc,
                                           in1=x3, op0=ALU.mult, op1=ALU.add)
            nc.sync.dma_start(out=out_d[b], in_=osb)


_CACHE = {}


def _get_compiled(debug=False):
    key = ("nc", debug)
    if key in _CACHE:
        return _CACHE[key]
    import concourse.bacc as bacc

    nc = bacc.Bacc("TRN2", target_bir_lowering=False, debug=False,
                   enable_asserts=False, num_devices=N_CORES)
    _build(nc, debug=debug)
    nc.compile()
    _CACHE[key] = nc
    return nc


def _run(inputs, trace=False, debug=False, **kw):
    from concourse import bass_utils

    nc = _get_compiled(debug=debug)
    x = np.ascontiguousarray(np.asarray(inputs["x"], dtype=np.float32))
    x = x.reshape(N, C, TV)
    f = lambda a: np.asarray(a, dtype=np.float32)
    R = C // 16
    w2t = np.zeros((IC, C), np.float32)
    w2t[0:R] = f(inputs["W2"]).T
    b1c = np.zeros((IC, 1), np.float32)
    b1c[0:R, 0] = f(inputs["b1"]).reshape(R)
    wp32 = np.ascontiguousarray(np.concatenate([
        f(inputs["Wq"]),                       # 0:64
        f(inputs["Wk"]),                       # 64:128
        f(inputs["bk"]).reshape(IC, 1),        # 128
        np.zeros((IC, 1), np.float32),         # 129 (pad)
        f(inputs["Wv"]),                       # 130:194
        f(inputs["bv"]).reshape(IC, 1),        # 194
        f(inputs["Wt"]).T,                     # 195:259  Wt^T
        w2t,                                   # 259:323  W2^T (rows 0:4)
        b1c,                                   # 323
    ], axis=1))
    wp64 = np.ascontiguousarray(np.concatenate([
        f(inputs["W1"]).T,                     # 0:4  W1^T
        f(inputs["gamma"]).reshape(C, 1),
        f(inputs["beta"]).reshape(C, 1),
        f(inputs["b2"]).reshape(C, 1),
    ], axis=1))
    common = {"wp32": wp32, "wp64": wp64}
    in_maps = []
    for c in range(N_CORES):
        m = dict(common)
        m["x_in"] = np.ascontiguousarray(x[c * NB:(c + 1) * NB])
        in_maps.append(m)
    try:
        res = bass_utils.run_bass_kernel_spmd(
            nc, in_maps, core_ids=list(range(N_CORES)), trace=trace, **kw)
    except Exception:
        import time as _time
        _time.sleep(5)
        res = bass_utils.run_bass_kernel_spmd(
            nc, in_maps, core_ids=list(range(N_CORES)), trace=False, **kw)
    out = np.concatenate([res.results[c]["out"] for c in range(N_CORES)], axis=0)
    return out.reshape(N, C, T, V).astype(np.float32), res


def kernel(**inputs):
    return _run(inputs, trace=False)[0]


# revision 44
# speedup vs baseline: 1.0694x; 1.0666x over previous
"""Trainium2 Bass kernel for nn_FEM_35072702939287 (attention + BN + channel gate).

Math restructuring (validated vs reference):
  A[t,s] = (Wk x + bk)[:,t] . (Wq x + bq)[:,s]
         = [X_aug^T @ H_aug](t,s) + row-const(t) + const
  where X_aug = [X; 1] (65 x TV), H_aug = [G X ; r^T X], G = Wk^T Wq,
  r = Wq^T bk.  Row-constant terms drop under softmax over s.
  We compute A^T tiles [s_block=128, t] so softmax's denominator
  D[t] = sum_s exp(A^T[s,t]) falls out of the PV matmul by augmenting
  the value projection with a ones column.

  The Trans_s conv is folded into V before attention:
  u[s,c] = sum_i v[s,i] Wt[c,i] = X_aug^T @ [WC; bc],  WC = (Wt Wv)^T,
  bc = Wt bv  (the conv bias bt cancels under BN), so the PV
  accumulation directly yields [P2~ ; D] (65 rows x TV) and the old
  post-attention Wt matmuls disappear.  1/D reaches all 64 channel rows
  via a ones-column fp32r replication matmul.  All fp16 matmuls move
  800 columns per instruction (PSUM allows two-bank outputs; staying at
  512 doubles the per-matmul ~219-cycle drain overhead).
  BatchNorm batch stats go through a small AllGather (lower latency
  floor than AllReduce); each core averages the 8 [mean, E[x^2]] pairs.

Sharding: data-parallel over batch N=16 -> 2 batches per core x 8 cores.
"""

import os
import numpy as np

N_CORES = 8
N, C, T, V = 16, 64, 64, 25
TV = T * V            # 1600
IC = 32
NB = N // N_CORES     # batches per core
EPS = 1e-5
NSB = 13              # 12 full 128-row s-blocks + one 64-row tail
SB = [(j * 128, 128) for j in range(12)] + [(1536, 64)]
# phase1 A-psum half-tiles [128, 800] (2 banks); matmul outputs are
# capped at one PSUM bank (512 fp32), so 800-wide tiles split 512+288
HALVES = [(0, 800), (800, 1600)]
CH_H = [(0, 512), (512, 800)]
# phase2: two 800-wide accumulators
CH_P = [(0, 800), (800, 1600)]
# fp32 sub-chunks within an 800-wide psum tile (fp32 moving max is 512)
CH_R = [(0, 512), (512, 800)]
UW = C + 2            # u-projection width: 64 ch + D ones col + pad


def _build(nc, debug=False):
    import concourse.tile as tile
    from concourse import mybir
    from contextlib import ExitStack

    f32 = mybir.dt.float32
    f32r = mybir.dt.float32r
    f16 = mybir.dt.float16
    AF = mybir.ActivationFunctionType
    ALU = mybir.AluOpType
    AX = mybir.AxisListType

    # ---------------- DRAM I/O ----------------
    # weights are host-packed into two tensors so startup needs only two
    # small DMAs (each extra DMA costs ~1.3us of issue latency in series)
    x_in = nc.dram_tensor("x_in", [NB, C, TV], f32, kind="ExternalInput").ap()
    wp32_d = nc.dram_tensor("wp32", [IC, 324], f32, kind="ExternalInput").ap()
    wp64_d = nc.dram_tensor("wp64", [C, 7], f32, kind="ExternalInput").ap()
    out_d = nc.dram_tensor("out", [NB, C, TV], f32, kind="ExternalOutput").ap()
    if debug:
        dbg_ha = nc.dram_tensor("dbg_ha", [C + 1, TV], f32, kind="ExternalOutput").ap()
        dbg_ut = nc.dram_tensor("dbg_ut", [128, NSB, UW], f16, kind="ExternalOutput").ap()
        dbg_eb = nc.dram_tensor("dbg_eb", [128, NSB, TV], f16, kind="ExternalOutput").ap()
        dbg_p2 = nc.dram_tensor("dbg_p2", [NB, C, TV], f32, kind="ExternalOutput").ap()
        dbg_gate = nc.dram_tensor("dbg_gate", [C, NB], f32, kind="ExternalOutput").ap()
        dbg_scsh = nc.dram_tensor("dbg_scsh", [C, 2], f32, kind="ExternalOutput").ap()

    R = C // 16  # 4

    with tile.TileContext(nc) as tc, ExitStack() as ctx:
        consts = ctx.enter_context(tc.tile_pool(name="consts", bufs=1))
        xpool = ctx.enter_context(tc.tile_pool(name="xpool", bufs=2))
        workp = ctx.enter_context(tc.tile_pool(name="workp", bufs=2))
        finp = ctx.enter_context(tc.tile_pool(name="finp", bufs=1))
        statp = ctx.enter_context(tc.tile_pool(name="statp", bufs=1))
        psA = ctx.enter_context(tc.tile_pool(name="psA", bufs=2, space="PSUM"))
        psW = ctx.enter_context(tc.tile_pool(name="psW", bufs=2, space="PSUM"))
        dramp = ctx.enter_context(tc.tile_pool(name="dramp", bufs=1, space="DRAM"))

        # Fire-and-forget collective warmup: initializes the CC channel so
        # the real stats collective later starts with minimal setup cost.
        warm_ci = dramp.tile([1, 2], f32, name="warm_ci")
        warm_co = dramp.tile([N_CORES, 2], f32, name="warm_co", addr_space="Shared")
        nc.gpsimd.collective_compute(
            "AllGather",
            ALU.bypass,
            ins=[warm_ci.opt()],
            outs=[warm_co.opt()],
            replica_groups=[list(range(N_CORES))],
        )

        # ---------------- input DMAs first (sync queue is the x path) ----------
        xa = [None] * NB      # [65, TV] f32 : [X; 1]
        for b in range(NB):
            t = xpool.tile([C + 1, TV], f32, name="xa", tag="xa")
            xa[b] = t
            nc.gpsimd.memset(t[C:C + 1, :], 1.0)
        # packed weights first (they gate the G/WC prep matmuls), then x
        wpack = consts.tile([IC, 324], f32)
        nc.sync.dma_start(out=wpack, in_=wp32_d)
        nc.sync.dma_start(out=xa[0][0:C, :], in_=x_in[0])
        wpk2 = consts.tile([C, 7], f32)
        nc.sync.dma_start(out=wpk2, in_=wp64_d)
        nc.sync.dma_start(out=xa[1][0:C, :], in_=x_in[1])

        # views into the packed weights
        wq_sb = wpack[:, 0:C]
        wkbk = wpack[:, C:2 * C + 2]               # [Wk | bk | 0]
        wvb = wpack[:, 2 * C + 2:3 * C + 3]        # [Wv | bv]
        wt_rep = wpack[:, 3 * C + 3:4 * C + 3]     # Wt^T
        w2t = wpack[0:R, 4 * C + 3:5 * C + 3]      # W2^T (rows 0:4)
        b1_sb = wpack[0:R, 5 * C + 3:5 * C + 4]
        w1t = wpk2[:, 0:R]                         # W1^T
        gamma_sb = wpk2[:, R:R + 1]
        beta_sb = wpk2[:, R + 1:R + 2]
        b2_sb = wpk2[:, R + 2:R + 3]

        ones1f = consts.tile([C + 1, C], f32)
        nc.vector.memset(ones1f, 1.0)
        # warm up the ACT table (exp set also holds Identity)
        warmz = consts.tile([1, 1], f32)
        nc.vector.memset(warmz, 1.0)
        warmo = consts.tile([1, 1], f32)
        nc.scalar.activation(warmo, warmz, AF.Exp)
        magic = consts.tile([C, 1], mybir.dt.int32)
        nc.vector.memset(magic, 0x5F3759DF)
        b2n = consts.tile([C, 1], f32)
        nc.vector.tensor_scalar_mul(b2n, b2_sb, -1.0)

        # G^T | r  =  Wq^T @ [Wk | bk]   -> lhsT for the H matmul
        psg = psW.tile([C, C + 2], f32, name="psg", tag="w")
        nc.tensor.matmul(psg, lhsT=wq_sb, rhs=wkbk, start=True, stop=True)
        gr = consts.tile([C, C + 1], f16)
        nc.vector.tensor_copy(gr, psg[:, 0:C + 1])

        # [WC; bc] = [Wv | bv]^T @ Wt^T  -> rhs for the u-projection
        pswc = psW.tile([C + 1, C], f32, name="pswc", tag="w")
        nc.tensor.matmul(pswc, lhsT=wvb, rhs=wt_rep, start=True, stop=True)
        wct_f = consts.tile([C + 1, UW], f32)
        nc.vector.memset(wct_f, 0.0)
        nc.vector.tensor_copy(wct_f[:, 0:C], pswc)
        nc.vector.memset(wct_f[C:C + 1, C:C + 1], 1.0)
        wct = consts.tile([C + 1, UW], f16)
        nc.vector.tensor_copy(wct, wct_f)

        # ---------------- per-batch state ----------------
        xr = [None] * NB      # [65, TV] f16 copy for matmul operands
        ha = [None] * NB      # [65, TV] f16 : [G X; r^T X]
        ut1 = [None] * NB     # [128, 13, 66] f16 : [U | 1 | 0] per s-block
        eb = [None] * NB      # [128, 13, TV] f16 : exp(A^T)
        p2 = [None] * NB      # [64, TV] f32 : p2 (pre-BN, post conv+divide)
        avgs = statp.tile([C, NB], f32)
        stats = statp.tile([C, NB * len(CH_P) * 2, 6], f32)

        def prologue(b, cast_on_scalar=False):
            t = xa[b]
            tr = xpool.tile([C + 1, TV], f16, name="xr", tag="xr")
            xr[b] = tr
            if cast_on_scalar:
                # DVE is the startup bottleneck; ScE is idle before phase1
                nc.scalar.activation(tr, t, AF.Identity)
            else:
                nc.vector.tensor_copy(tr, t)
            h = xpool.tile([C + 1, TV], f16, name="ha", tag="ha")
            ha[b] = h
            for (t0, t1) in CH_P:
                hps = psW.tile([C + 1, 800], f32, name="hps", tag="w")
                for (c0, c1) in CH_H:
                    nc.tensor.matmul(hps[:, c0:c1], lhsT=gr,
                                     rhs=tr[0:C, t0 + c0:t0 + c1],
                                     start=True, stop=True)
                nc.vector.tensor_copy(h[:, t0:t1], hps)
            ut1[b] = xpool.tile([128, NSB, UW], f16, name="ut1", tag="ut1")
            eb[b] = xpool.tile([128, NSB, TV], f16, name="eb", tag="eb")
            p2[b] = xpool.tile([C, TV], f32, name="p2", tag="p2")
            # all u-projection blocks up front (keeps phase1 PE-dense)
            for j, (off, p) in enumerate(SB):
                vps = psW.tile([128, UW], f32, name="vps", tag="w")
                nc.tensor.matmul(vps[0:p, :], lhsT=tr[:, off:off + p],
                                 rhs=wct, start=True, stop=True)
                nc.vector.tensor_copy(ut1[b][0:p, j, :], vps[0:p, :])
            nc.vector.reduce_sum(avgs[:, b:b + 1], t[0:C, :], axis=AX.X)

        def pv_mm(b, paccs, j, ti):
            off, p = SB[j]
            t0, t1 = CH_P[ti]
            for (c0, c1) in CH_H:
                nc.tensor.matmul(paccs[ti][0:UW, c0:c1],
                                 lhsT=ut1[b][0:p, j, :],
                                 rhs=eb[b][0:p, j, t0 + c0:t0 + c1],
                                 start=(j == 0), stop=(j == NSB - 1))

        def phase1(b, inject=None):
            """A^T block -> exp -> PV accumulation.  The second-half PV of
            block j is emitted after A of block j+1 so its wait on exp(j,h2)
            never blocks the in-order PE queue.  `inject` maps j -> thunk
            emitted at that iteration (fills PE bubbles with foreign work)."""
            paccs = []
            for ti, (t0, t1) in enumerate(CH_P):
                paccs.append(psW.tile([UW, 800], f32, name=f"pacc{ti}",
                                      tag="w"))
            for j, (off, p) in enumerate(SB):
                for (h0, h1) in HALVES:
                    aps = psA.tile([128, 800], f32, name="aps", tag="aps")
                    for (c0, c1) in CH_H:
                        nc.tensor.matmul(aps[0:p, c0:c1],
                                         lhsT=ha[b][:, off:off + p],
                                         rhs=xr[b][:, h0 + c0:h0 + c1],
                                         start=True, stop=True)
                    nc.scalar.activation(eb[b][0:p, j, h0:h1], aps[0:p, :], AF.Exp)
                if inject and j in inject:
                    inject[j]()
                if j > 0:
                    pv_mm(b, paccs, j - 1, 1)
                pv_mm(b, paccs, j, 0)
            pv_mm(b, paccs, NSB - 1, 1)
            return paccs

        def remainder(b, paccs):
            """[P2~ ; D] -> 1/D replication -> divide -> bn_stats per chunk.
            The psum->SBUF D copies run on ScE (idle after the last exp)."""
            pds = []
            for ti, (t0, t1) in enumerate(CH_P):
                pd = workp.tile([C + 1, 800], f32, name="pd", tag="pd")
                pds.append(pd)
                nc.scalar.activation(pd, paccs[ti][0:C + 1, :], AF.Identity)
            for ti, (t0, t1) in enumerate(CH_P):
                w = t1 - t0
                pd = pds[ti]
                dps = psW.tile([C, 800], f32, name="dps", tag="w")
                for (c0, c1) in CH_R:
                    nc.tensor.matmul(dps[:, c0:c1], lhsT=ones1f[C:C + 1, :],
                                     rhs=pd[C:C + 1, c0:c1],
                                     start=True, stop=True)
                rrep = workp.tile([C, 800], f32, name="rrep", tag="rrep")
                nc.vector.reciprocal_approx_fast(out=rrep[:, 0:w], in_=dps[:, 0:w])
                nc.vector.tensor_mul(p2[b][:, t0:t1], pd[0:C, 0:w],
                                     rrep[:, 0:w])
                nc.vector.bn_stats(stats[:, 2 * (b * len(CH_P) + ti), :],
                                   p2[b][:, t0:t0 + 512])
                nc.vector.bn_stats(stats[:, 2 * (b * len(CH_P) + ti) + 1, :],
                                   p2[b][:, t0 + 512:t1])

        prologue(0, cast_on_scalar=True)
        pa0 = phase1(0, inject={3: lambda: prologue(1)})
        remainder(0, pa0)

        # ---------------- channel gate (hidden under phase1(1)) ----------------
        hps2 = psW.tile([R, NB], f32, name="hps2", tag="w")
        nc.tensor.matmul(hps2, lhsT=w1t, rhs=avgs, start=True, stop=True)
        h_pre = statp.tile([R, NB], f32)
        nc.vector.tensor_scalar(h_pre, hps2, 1.0 / TV, b1_sb,
                                op0=ALU.mult, op1=ALU.add)
        h_sb = statp.tile([R, NB], f32)
        nc.vector.tensor_scalar_max(h_sb, h_pre, 0.0)
        zps = psW.tile([C, NB], f32, name="zps", tag="w")
        nc.tensor.matmul(zps, lhsT=w2t, rhs=h_sb, start=True, stop=True)
        eg = statp.tile([C, NB], f32)
        nc.scalar.activation(eg, zps, AF.Exp, bias=b2n, scale=-1.0)
        gp1 = statp.tile([C, NB], f32)
        nc.vector.tensor_scalar_add(gp1, eg, 1.0)
        gate = statp.tile([C, NB], f32)
        nc.vector.reciprocal(gate, gp1)

        # w_b = gate (.) p2_b can be computed before the stats collective
        wts = [None] * NB

        def w_precompute(b):
            u = workp.tile([C, TV], f32, name="u", tag="u")
            wts[b] = u
            nc.vector.tensor_scalar_mul(u, p2[b], gate[:, b:b + 1])

        w_precompute(0)
        pa1 = phase1(1)
        remainder(1, pa1)
        w_precompute(1)

        # ---------------- BN stats: local -> allgather -> global ----------------
        mv = statp.tile([C, 2], f32)
        nc.vector.bn_aggr(out=mv, in_=stats)
        m2 = statp.tile([C, 1], f32)
        nc.vector.tensor_mul(m2, mv[:, 0:1], mv[:, 0:1])
        sums = statp.tile([C, 2], f32)
        nc.vector.tensor_copy(sums[:, 0:1], mv[:, 0:1])
        nc.vector.tensor_add(sums[:, 1:2], mv[:, 1:2], m2)

        cc_in = dramp.tile([C, 2], f32, name="cc_in")
        cc_out = dramp.tile([N_CORES, C, 2], f32, name="cc_out",
                            addr_space="Shared")
        nc.sync.dma_start(out=cc_in, in_=sums)
        nc.gpsimd.collective_compute(
            "AllGather",
            ALU.bypass,
            ins=[cc_in.opt()],
            outs=[cc_out.opt()],
            replica_groups=[list(range(N_CORES))],
        )
        gs8 = statp.tile([C, 2, N_CORES], f32)
        nc.sync.dma_start(out=gs8, in_=cc_out[:, :, :].rearrange("r c k -> c k r"))

        # Work on 8x-scaled sums to skip the mean/var normalization ops:
        #   v64 = 64*var = (8*gsum1 + 64*eps) - gsum0^2
        #   rstd64 = rsqrt(v64) = rstd/8 ;  sc = (8*gamma)*rstd64
        #   sh = beta - mean*sc = beta - gsum0*(gamma*rstd64)
        gsum = statp.tile([C, 2], f32)
        nc.vector.reduce_sum(gsum, gs8, axis=AX.X)
        mg2 = statp.tile([C, 1], f32)
        nc.vector.tensor_mul(mg2, gsum[:, 0:1], gsum[:, 0:1])
        q8 = statp.tile([C, 1], f32)
        nc.vector.tensor_scalar(q8, gsum[:, 1:2], float(N_CORES),
                                float(N_CORES * N_CORES) * EPS,
                                op0=ALU.mult, op1=ALU.add)
        ve = statp.tile([C, 1], f32)
        nc.vector.tensor_sub(ve, q8, mg2)
        # rstd via fast-inverse-sqrt + 2 Newton steps (no ACT table switch)
        hsh = statp.tile([C, 1], mybir.dt.int32)
        nc.vector.tensor_scalar(hsh, ve.bitcast(mybir.dt.int32), 1, None,
                                op0=ALU.arith_shift_right)
        yi = statp.tile([C, 1], mybir.dt.int32)
        nc.vector.tensor_sub(yi, magic, hsh)
        r1 = statp.tile([C, 1], f32)
        rstd = statp.tile([C, 1], f32)
        t1 = statp.tile([C, 1], f32)
        t3 = statp.tile([C, 1], f32)
        y = yi.bitcast(f32)
        for it, dst in ((0, r1), (1, rstd)):
            nc.vector.tensor_mul(t1, y, y)
            nc.vector.tensor_mul(t1, t1, ve)
            nc.vector.tensor_scalar(t3, t1, -0.5, 1.5, op0=ALU.mult, op1=ALU.add)
            nc.vector.tensor_mul(dst, y, t3)
            y = dst
        scq = statp.tile([C, 1], f32)
        nc.vector.tensor_mul(scq, gamma_sb, rstd)
        sc = statp.tile([C, 1], f32)
        nc.vector.tensor_scalar_mul(sc, scq, float(N_CORES))
        msc = statp.tile([C, 1], f32)
        nc.vector.tensor_mul(msc, gsum[:, 0:1], scq)
        sh = statp.tile([C, 1], f32)
        nc.vector.tensor_sub(sh, beta_sb, msc)

        if debug:
            nc.sync.dma_start(out=dbg_ha, in_=ha[0].bitcast(f32))
            nc.sync.dma_start(out=dbg_ut, in_=ut1[0])
            nc.sync.dma_start(out=dbg_eb, in_=eb[0])
            for _b in range(NB):
                nc.sync.dma_start(out=dbg_p2[_b], in_=p2[_b])
            nc.sync.dma_start(out=dbg_gate, in_=gate)
            nc.sync.dma_start(out=dbg_scsh[:, 0:1], in_=sc)
            nc.sync.dma_start(out=dbg_scsh[:, 1:2], in_=sh)

        # ------------- finalize: out = sc*(gate*p2) + (x + gate*sh) ------------
        # batch 0 on Scalar+Vector, batch 1 on GpSimd, concurrently
        d_0 = statp.tile([C, 1], f32, name="d_0")
        nc.vector.tensor_mul(d_0, gate[:, 0:1], sh)
        d_1 = statp.tile([C, 1], f32, name="d_1")
        nc.vector.tensor_mul(d_1, gate[:, 1:2], sh)
        # chunked so the first osb half starts as soon as the first x3 half
        # lands (ScE and DVE pipeline at half-tile granularity)
        for b, d_b in ((0, d_0), (1, d_1)):
            x3 = finp.tile([C, TV], f32, name=f"x3_{b}", tag=f"x3_{b}")
            osb = finp.tile([C, TV], f32, name=f"osb_{b}", tag=f"osb_{b}")
            for (t0, t1) in CH_P:
                nc.scalar.activation(x3[:, t0:t1], xa[b][0:C, t0:t1],
                                     AF.Identity, bias=d_b)
                nc.vector.scalar_tensor_tensor(out=osb[:, t0:t1],
                                               in0=wts[b][:, t0:t1], scalar=sc,
                                               in1=x3[:, t0:t1],
                                               op0=ALU.mult, op1=ALU.add)
            nc.sync.dma_start(out=out_d[b], in_=osb)


_CACHE = {}


def _get_compiled(debug=False):
    key = ("nc", debug)
    if key in _CACHE:
        return _CACHE[key]
    import concourse.bacc as bacc

    nc = bacc.Bacc("TRN2", target_bir_lowering=False, debug=False,
                   enable_asserts=False, num_devices=N_CORES)
    _build(nc, debug=debug)
    nc.compile()
    _CACHE[key] = nc
    return nc


def _run(inputs, trace=False, debug=False, **kw):
    from concourse import bass_utils

    nc = _get_compiled(debug=debug)
    x = np.ascontiguousarray(np.asarray(inputs["x"], dtype=np.float32))
    x = x.reshape(N, C, TV)
    f = lambda a: np.asarray(a, dtype=np.float32)
    R = C // 16
    w2t = np.zeros((IC, C), np.float32)
    w2t[0:R] = f(inputs["W2"]).T
    b1c = np.zeros((IC, 1), np.float32)
    b1c[0:R, 0] = f(inputs["b1"]).reshape(R)
    wp32 = np.ascontiguousarray(np.concatenate([
        f(inputs["Wq"]),                       # 0:64
        f(inputs["Wk"]),                       # 64:128
        f(inputs["bk"]).reshape(IC, 1),        # 128
        np.zeros((IC, 1), np.float32),         # 129 (pad)
        f(inputs["Wv"]),                       # 130:194
        f(inputs["bv"]).reshape(IC, 1),        # 194
        f(inputs["Wt"]).T,                     # 195:259  Wt^T
        w2t,                                   # 259:323  W2^T (rows 0:4)
        b1c,                                   # 323
    ], axis=1))
    wp64 = np.ascontiguousarray(np.concatenate([
        f(inputs["W1"]).T,                     # 0:4  W1^T
        f(inputs["gamma"]).reshape(C, 1),
        f(inputs["beta"]).reshape(C, 1),
        f(inputs["b2"]).reshape(C, 1),
    ], axis=1))
    common = {"wp32": wp32, "wp64": wp64}
    in_maps = []
    for c in range(N_CORES):
        m = dict(common)
        m["x_in"] = np.ascontiguousarray(x[c * NB:(c + 1) * NB])
        in_maps.append(m)
    try:
        res = bass_utils.run_bass_kernel_spmd(
            nc, in_maps, core_ids=list(range(N_CORES)), trace=trace, **kw)
    except Exception:
        import time as _time
        _time.sleep(5)
        res = bass_utils.run_bass_kernel_spmd(
            nc, in_maps, core_ids=list(range(N_CORES)), trace=False, **kw)
    out = np.concatenate([res.results[c]["out"] for c in range(N_CORES)], axis=0)
    return out.reshape(N, C, T, V).astype(np.float32), res


def kernel(**inputs):
    return _run(inputs, trace=False)[0]
